# Initial kernel scaffold
#
"""Trainium2 Bass kernel for MemoryOptimizedAttention (MHA with projections).

Problem (hardcoded): B=4, T=2048, D=1024, H=16, DH=64, fp32 I/O.

Sharding: 8 cores = (batch b, T-half) pairs. Each core computes the full
attention output for its 1024 query rows of batch b (all 16 heads), using
the full 2048-key context of that batch. No collectives; host gathers.

Device dataflow (feature-major / transposed layouts throughout):
  QT = Wq @ xqT (+bq)            [1024, 1024]   per head-pair (hp) chunks
  KT = Wk @ xkT (+bk)            [1024, 2048]
  V' = xvT.T @ WvT               [2048, 130/hp] k-major, with ones cols
  S^T[k,q] = KT_h.T-slice @ QT_h [128, 512] tiles -> exp on ScalarE
  O'^T = V'^T @ attn^T           [65, 512]  (row 64 = softmax denominator)
  O^T = O'^T[0:64] * bcast(1/denom)
  Y^T = Wo @ O^T                 [1024, 1024]
bv and bo are folded on the host into a constant row added to the output.
"""

import sys

for _p in ("/opt/trn_rl_repo",):
    if _p not in sys.path:
        sys.path.insert(0, _p)

import numpy as np

import concourse.bass as bass
import concourse.mybir as mybir
import concourse.tile as tile
from concourse import bacc
from concourse import bass_utils
from concourse.bass import ts, ds

B, T, D, H = 4, 2048, 1024, 16
DH = D // H
SCALE = 1.0 / float(np.sqrt(DH))

P = 128
HP = 8     # head pairs
CC = 8     # 128-wide chunks of D
KC = 16    # 128-wide chunks of the key/context dim (2048)
QT = 2     # 512-wide q tiles per core (Tq = 1024)
KT4 = 4    # k-chunks per S^T psum supertile
F = 512
TQ = 1024  # q rows per core
TK = 2048  # context rows per core

fp16 = mybir.dt.float16
f32 = mybir.dt.float32
EXP = mybir.ActivationFunctionType.Exp

N_CORES = 8


def build_nc():
    nc = bacc.Bacc(None, target_bir_lowering=False, debug=False)

    xq = nc.dram_tensor("xq", [P, CC, TQ], fp16, kind="ExternalInput")
    xk = nc.dram_tensor("xk", [P, CC, TK], fp16, kind="ExternalInput")
    xv = nc.dram_tensor("xv", [P, CC, TK], fp16, kind="ExternalInput")
    wq = nc.dram_tensor("wq", [P, HP, CC, P], fp16, kind="ExternalInput")
    wk = nc.dram_tensor("wk", [P, HP, CC, P], fp16, kind="ExternalInput")
    wv = nc.dram_tensor("wv", [P, HP, CC, P], fp16, kind="ExternalInput")
    wo = nc.dram_tensor("wo", [DH, CC, H, P], fp16, kind="ExternalInput")
    bq = nc.dram_tensor("bq", [P, HP], f32, kind="ExternalInput")
    bk = nc.dram_tensor("bk", [P, HP], f32, kind="ExternalInput")
    yT = nc.dram_tensor("yT", [P, CC, TQ], f32, kind="ExternalOutput")

    with tile.TileContext(nc) as tc:
        with (
            tc.tile_pool(name="res", bufs=1) as res,
            tc.tile_pool(name="wpool", bufs=2) as wpool,
            tc.tile_pool(name="hpp", bufs=2) as hpp,
            tc.tile_pool(name="apool", bufs=2) as apool,
            tc.tile_pool(name="npool", bufs=2) as npool,
            tc.tile_pool(name="mmp", bufs=2, space="PSUM") as mmp,
            tc.tile_pool(name="stp", bufs=1, space="PSUM") as stp,
            tc.tile_pool(name="avp", bufs=2, space="PSUM") as avp,
        ):
            xq_sb = res.tile([P, CC, TQ], fp16)
            nc.sync.dma_start(xq_sb[:], xq[:])
            xk_sb = res.tile([P, CC, TK], fp16)
            nc.sync.dma_start(xk_sb[:], xk[:])
            xv_sb = res.tile([P, CC, TK], fp16)
            nc.sync.dma_start(xv_sb[:], xv[:])
            bq_sb = res.tile([P, HP], f32)
            nc.sync.dma_start(bq_sb[:], bq[:])
            bk_sb = res.tile([P, HP], f32)
            nc.sync.dma_start(bk_sb[:], bk[:])
            ot_sb = res.tile([DH, H, TQ], fp16)

            for hp in range(HP):
                wq_t = wpool.tile([P, CC, P], fp16, tag="wq")
                nc.sync.dma_start(wq_t[:], wq[:, hp])
                wk_t = wpool.tile([P, CC, P], fp16, tag="wk")
                nc.sync.dma_start(wk_t[:], wk[:, hp])
                wv_t = wpool.tile([P, CC, P], fp16, tag="wv")
                nc.sync.dma_start(wv_t[:], wv[:, hp])

                qt_sb = hpp.tile([P, TQ], fp16, tag="qt")
                kt_sb = hpp.tile([P, TK], fp16, tag="kt")
                vp_sb = hpp.tile([P, KC, 130], fp16, tag="vp")
                # ones columns (64 and 129) for the softmax denominator
                nc.vector.memset(
                    vp_sb[:].rearrange("p kc (g j) -> p kc g j", g=2)[:, :, :, 64:65],
                    1.0,
                )

                for t in range(QT):
                    ps = mmp.tile([P, F], f32, tag="proj")
                    for c in range(CC):
                        nc.tensor.matmul(
                            ps[:],
                            wq_t[:, c, :],
                            xq_sb[:, c, ts(t, F)],
                            start=(c == 0),
                            stop=(c == CC - 1),
                        )
                    nc.vector.tensor_scalar_add(
                        qt_sb[:, ts(t, F)], ps[:], bq_sb[:, hp : hp + 1]
                    )

                for t in range(TK // F):
                    ps = mmp.tile([P, F], f32, tag="proj")
                    for c in range(CC):
                        nc.tensor.matmul(
                            ps[:],
                            wk_t[:, c, :],
                            xk_sb[:, c, ts(t, F)],
                            start=(c == 0),
                            stop=(c == CC - 1),
                        )
                    nc.vector.tensor_scalar_add(
                        kt_sb[:, ts(t, F)], ps[:], bk_sb[:, hp : hp + 1]
                    )

                for kc in range(KC):
                    ps = mmp.tile([P, P], f32, tag="proj")
                    for c in range(CC):
                        nc.tensor.matmul(
                            ps[:],
                            xv_sb[:, c, ts(kc, P)],
                            wv_t[:, c, :],
                            start=(c == 0),
                            stop=(c == CC - 1),
                        )
                    nc.vector.tensor_copy(
                        vp_sb[:, kc].rearrange("p (g j) -> p g j", g=2)[:, :, 0:64],
                        ps[:].rearrange("p (g j) -> p g j", g=2),
                    )

                for h in range(2):
                    pb = DH * h
                    s = 2 * hp + h
                    for t in range(QT):
                        av = avp.tile([P, F], f32, tag="av")
                        for r in range(KC // KT4):
                            st = stp.tile([P, KT4, F], f32, tag="st")
                            for i in range(KT4):
                                kc = r * KT4 + i
                                nc.tensor.matmul(
                                    st[:, i, :],
                                    kt_sb[pb : pb + DH, ts(kc, P)],
                                    qt_sb[pb : pb + DH, ts(t, F)],
                                    start=True,
                                    stop=True,
                                )
                            at = apool.tile([P, KT4, F], fp16, tag="attn")
                            nc.scalar.activation(at[:], st[:], EXP, scale=SCALE)
                            for i in range(KT4):
                                kc = r * KT4 + i
                                nc.tensor.matmul(
                                    av[0:65, :],
                                    vp_sb[:, kc, ds(65 * h, 65)],
                                    at[:, i, :],
                                    start=(kc == 0),
                                    stop=(kc == KC - 1),
                                )
                        rd = npool.tile([P, F], f32, tag="rd")
                        nc.vector.reciprocal(rd[64:65, :], av[64:65, :])
                        bc = npool.tile([DH, F], f32, tag="bc")
                        nc.gpsimd.partition_broadcast(bc[:], rd[64:65, :])
                        nc.vector.tensor_mul(
                            ot_sb[:, s, ts(t, F)], av[0:DH, :], bc[:]
                        )

            for dc in range(CC):
                wo_t = wpool.tile([DH, H, P], fp16, tag="wo")
                nc.sync.dma_start(wo_t[:], wo[:, dc])
                ysb = npool.tile([P, TQ], f32, tag="y")
                for t in range(QT):
                    ps = mmp.tile([P, F], f32, tag="proj")
                    for s in range(H):
                        nc.tensor.matmul(
                            ps[:],
                            wo_t[:, s, :],
                            ot_sb[:, s, ts(t, F)],
                            start=(s == 0),
                            stop=(s == H - 1),
                        )
                    nc.vector.tensor_copy(ysb[:, ts(t, F)], ps[:])
                nc.sync.dma_start(yT[:, dc, :], ysb[:])

    nc.compile()
    return nc


_NC_CACHE = None


def _get_nc():
    global _NC_CACHE
    if _NC_CACHE is None:
        _NC_CACHE = build_nc()
    return _NC_CACHE


def _chunk_T(a):
    # [rows, D] f32 -> [P, D//P, rows] fp16 (feature-major chunks)
    return np.ascontiguousarray(
        a.T.reshape(CC, P, a.shape[0]).transpose(1, 0, 2).astype(np.float16)
    )


def _prep_in_maps(query, key, value, Wq, bq, Wk, bk, Wv, bv, Wo, bo):
    wq_d = np.ascontiguousarray(
        Wq.reshape(HP, P, CC, P).transpose(3, 0, 2, 1).astype(np.float16)
    )
    wk_d = np.ascontiguousarray(
        Wk.reshape(HP, P, CC, P).transpose(3, 0, 2, 1).astype(np.float16)
    )
    wv_d = np.ascontiguousarray(
        Wv.reshape(HP, P, CC, P).transpose(3, 0, 2, 1).astype(np.float16)
    )
    wo_d = np.ascontiguousarray(
        Wo.reshape(CC, P, H, DH).transpose(3, 0, 2, 1).astype(np.float16)
    )
    bq_d = np.ascontiguousarray(bq.reshape(HP, P).T.astype(np.float32))
    bk_d = np.ascontiguousarray(bk.reshape(HP, P).T.astype(np.float32))

    in_maps = []
    for c in range(N_CORES):
        b, th = divmod(c, 2)
        in_maps.append(
            {
                "xq": _chunk_T(query[b, th * TQ : (th + 1) * TQ, :]),
                "xk": _chunk_T(key[b]),
                "xv": _chunk_T(value[b]),
                "wq": wq_d,
                "wk": wk_d,
                "wv": wv_d,
                "wo": wo_d,
                "bq": bq_d,
                "bk": bk_d,
            }
        )
    return in_maps


def _gather(results, bv, bo, Wo):
    crow = (bv.astype(np.float32) @ Wo.T.astype(np.float32) + bo).astype(np.float32)
    out = np.empty((B, T, D), np.float32)
    for c in range(N_CORES):
        b, th = divmod(c, 2)
        y = results[c]["yT"]  # [P, CC, TQ] f32 = Y^T chunks
        out[b, th * TQ : (th + 1) * TQ, :] = (
            y.transpose(1, 0, 2).reshape(D, TQ).T + crow
        )
    return out


def _run(inputs, trace=False, **kwargs):
    inputs = {k: np.asarray(v) for k, v in inputs.items()}
    nc = _get_nc()
    in_maps = _prep_in_maps(**inputs)
    res = bass_utils.run_bass_kernel_spmd(
        nc, in_maps, core_ids=list(range(N_CORES)), trace=trace, **kwargs
    )
    out = _gather(res.results, inputs["bv"], inputs["bo"], inputs["Wo"])
    return out, res


def kernel(**inputs):
    out, _ = _run(inputs, trace=False)
    return out


# revision 51
# speedup vs baseline: 75.4380x; 75.4380x over previous
"""Trainium2 Bass kernel for MemoryOptimizedAttention (MHA with projections).

Problem (hardcoded): B=4, T=2048, D=1024, H=16, DH=64, fp32 I/O.

Sharding: 8 cores = (batch b, T-half) pairs. Each core computes the full
attention output for its 1024 query rows of batch b (all 16 heads), using
the full 2048-key context of that batch. No collectives; host gathers.

Device dataflow (feature-major / transposed layouts throughout):
  QT = Wq @ xqT (+bq)            [1024, 1024]   per head-pair (hp) chunks
  KT = Wk @ xkT (+bk)            [1024, 2048]
  V' = xvT.T @ WvT               [2048, 130/hp] k-major, with ones cols
  S^T[k,q] = KT_h.T-slice @ QT_h [128, 512] tiles -> exp on ScalarE
  O'^T = V'^T @ attn^T           [65, 512]  (row 64 = softmax denominator)
  O^T = O'^T[0:64] * bcast(1/denom)
  Y^T = Wo @ O^T                 [1024, 1024]
bv and bo are folded on the host into a constant row added to the output.

Head pair (A at partitions 0-63, B at 64-127) S^T matmuls are emitted
adjacently so the PE runs them concurrently in disjoint row groups. The
next head-pair's projections are emission-interleaved with the current
pair's attention rounds so the PE has fill work while ScalarE runs exp.
"""

import sys

for _p in ("/opt/trn_rl_repo",):
    if _p not in sys.path:
        sys.path.insert(0, _p)

import numpy as np

import concourse.bass as bass
import concourse.mybir as mybir
import concourse.tile as tile
from concourse import bacc
from concourse import bass_utils
from concourse.bass import ts, ds

B, T, D, H = 4, 2048, 1024, 16
DH = D // H
SCALE = 1.0 / float(np.sqrt(DH))

P = 128
HP = 8     # head pairs
CC = 8     # 128-wide chunks of D
KC = 16    # 128-wide chunks of the key/context dim (2048)
QT = 2     # 512-wide q tiles per core (Tq = 1024)
F = 512
TQ = 1024  # q rows per core
TK = 2048  # context rows per core

fp16 = mybir.dt.float16
f32 = mybir.dt.float32
EXP = mybir.ActivationFunctionType.Exp

N_CORES = 8


def _interleave(a_items, b_items):
    """Emit two work-item lists interleaved evenly."""
    na, nb = len(a_items), len(b_items)
    ia = ib = 0
    while ia < na or ib < nb:
        if ia >= na:
            b_items[ib]()
            ib += 1
        elif ib >= nb:
            a_items[ia]()
            ia += 1
        elif ib * na <= ia * nb:
            b_items[ib]()
            ib += 1
        else:
            a_items[ia]()
            ia += 1


def build_nc(repeat=1):
    nc = bacc.Bacc(None, target_bir_lowering=False, debug=False)

    xq = nc.dram_tensor("xq", [P, CC, TQ], fp16, kind="ExternalInput")
    xk = nc.dram_tensor("xk", [P, CC, TK], fp16, kind="ExternalInput")
    xv = nc.dram_tensor("xv", [P, CC, TK], fp16, kind="ExternalInput")
    wq = nc.dram_tensor("wq", [P, HP, CC, P], fp16, kind="ExternalInput")
    wk = nc.dram_tensor("wk", [P, HP, CC, P], fp16, kind="ExternalInput")
    wv = nc.dram_tensor("wv", [P, CC, D], fp16, kind="ExternalInput")
    wo = nc.dram_tensor("wo", [DH, CC, H, P], fp16, kind="ExternalInput")
    bq = nc.dram_tensor("bq", [P, HP], f32, kind="ExternalInput")
    bk = nc.dram_tensor("bk", [P, HP], f32, kind="ExternalInput")
    yT = nc.dram_tensor("yT", [P, CC, TQ], f32, kind="ExternalOutput")

    with tile.TileContext(nc) as tc:
        with (
            tc.tile_pool(name="res", bufs=1) as res,
            tc.tile_pool(name="wpool", bufs=2) as wpool,
            tc.tile_pool(name="hpp", bufs=2) as hpp,
            tc.tile_pool(name="apool", bufs=2) as apool,
            tc.tile_pool(name="npool", bufs=2) as npool,
            tc.tile_pool(name="mmp", bufs=1, space="PSUM") as mmp,
            tc.tile_pool(name="stp", bufs=1, space="PSUM") as stp,
            tc.tile_pool(name="avp", bufs=2, space="PSUM") as avp,
            tc.tile_pool(name="bcp", bufs=1, space="PSUM") as bcp,
        ):
            ones_sb = res.tile([P, DH], fp16)
            nc.vector.memset(ones_sb[0:DH, :], 1.0)
            # exact-range write for the partition-64 bcast-matmul read
            nc.vector.memset(ones_sb[DH:P, :], 1.0)
            # half-partition DMAs so the base-64 bias reads in the split
            # projection evictions get dependency edges
            bq_sb = res.tile([P, HP], f32)
            nc.sync.dma_start(bq_sb[0:DH, :], bq[0:DH, :])
            nc.sync.dma_start(bq_sb[DH:P, :], bq[DH:P, :])
            bk_sb = res.tile([P, HP], f32)
            nc.sync.dma_start(bk_sb[0:DH, :], bk[0:DH, :])
            nc.sync.dma_start(bk_sb[DH:P, :], bk[DH:P, :])
            # resident x tiles; DMAs are emitted after the first head-pair's
            # weight loads (see below) so the first projections start early
            xq_sb = res.tile([P, CC, TQ], fp16)
            xk_sb = res.tile([P, CC, TK], fp16)
            xv_sb = res.tile([P, CC, TK], fp16)
            wv_sb = res.tile([P, CC, D], fp16)
            # V' for all head pairs, k-major, with ones cols at 64/129;
            # one tile per k-chunk keeps access patterns simple for the
            # dependency tracker
            vp_sbs = [
                res.tile([P, HP, 130], fp16, name=f"vp_sb{k}") for k in range(KC)
            ]

            def load_x():
                for c in range(CC):
                    nc.sync.dma_start(xq_sb[:, c], xq[:, c])
                for c in range(CC):
                    nc.sync.dma_start(xk_sb[:, c], xk[:, c])
                for c in range(CC):
                    nc.sync.dma_start(wv_sb[:, c], wv[:, c])
                    nc.sync.dma_start(xv_sb[:, c], xv[:, c])
                for k in range(KC):
                    nc.vector.memset(vp_sbs[k][:, :, 64:65], 1.0)
                    nc.vector.memset(vp_sbs[k][:, :, 129:130], 1.0)

            ot_sb = res.tile([DH, H, TQ], fp16)

            def vp_group(kc, half):
                # V' projection for 4 head pairs at once (512 output dims)
                ps = mmp.tile([P, F], f32, tag="proj", name="ps")
                for c in range(CC):
                    nc.tensor.matmul(
                        ps[:],
                        xv_sb[:, c, ts(kc, P)],
                        wv_sb[:, c, ts(half, F)],
                        start=(c == 0),
                        stop=(c == CC - 1),
                    )
                ps4 = ps[:].rearrange("p (hp g j) -> p hp g j", hp=4, g=2)
                nc.vector.tensor_copy(
                    vp_sbs[kc][:, 4 * half : 4 * half + 4, 0:64], ps4[:, :, 0, :]
                )
                nc.vector.tensor_copy(
                    vp_sbs[kc][:, 4 * half : 4 * half + 4, 65:129], ps4[:, :, 1, :]
                )

            def proj_items(hp):
                state = {}
                items = []

                def dma_weights():
                    wq_t = wpool.tile([P, CC, P], fp16, tag="wq", name="wq_t")
                    nc.sync.dma_start(wq_t[:], wq[:, hp])
                    wk_t = wpool.tile([P, CC, P], fp16, tag="wk", name="wk_t")
                    nc.sync.dma_start(wk_t[:], wk[:, hp])
                    qt_sb = hpp.tile([P, TQ], fp16, tag="qt", name="qt_sb")
                    kt_sb = hpp.tile([P, TK], fp16, tag="kt", name="kt_sb")
                    state.update(wq_t=wq_t, wk_t=wk_t, qt_sb=qt_sb, kt_sb=kt_sb)

                items.append(dma_weights)

                def qt_group(t):
                    ps = mmp.tile([P, F], f32, tag="proj", name="ps")
                    for c in range(CC):
                        nc.tensor.matmul(
                            ps[:],
                            state["wq_t"][:, c, :],
                            xq_sb[:, c, ts(t, F)],
                            start=(c == 0),
                            stop=(c == CC - 1),
                        )
                    # two half-partition writes so the dependency tracker
                    # links the head-B (partitions 64-127) S^T reads
                    nc.vector.tensor_scalar_add(
                        state["qt_sb"][0:DH, ts(t, F)],
                        ps[0:DH, :],
                        bq_sb[0:DH, hp : hp + 1],
                    )
                    nc.vector.tensor_scalar_add(
                        state["qt_sb"][DH:P, ts(t, F)],
                        ps[DH:P, :],
                        bq_sb[DH:P, hp : hp + 1],
                    )

                def kt_group(t):
                    ps = mmp.tile([P, F], f32, tag="proj", name="ps")
                    for c in range(CC):
                        nc.tensor.matmul(
                            ps[:],
                            state["wk_t"][:, c, :],
                            xk_sb[:, c, ts(t, F)],
                            start=(c == 0),
                            stop=(c == CC - 1),
                        )
                    nc.vector.tensor_scalar_add(
                        state["kt_sb"][0:DH, ts(t, F)],
                        ps[0:DH, :],
                        bk_sb[0:DH, hp : hp + 1],
                    )
                    nc.vector.tensor_scalar_add(
                        state["kt_sb"][DH:P, ts(t, F)],
                        ps[DH:P, :],
                        bk_sb[DH:P, hp : hp + 1],
                    )

                for t in range(QT):
                    items.append(lambda t=t: qt_group(t))
                for t in range(TK // F):
                    items.append(lambda t=t: kt_group(t))
                # V' groups (all-hp, 512-wide) distributed across the hp
                # pipeline just ahead of their consumers
                if hp == 0:
                    for kc in range(KC):
                        items.append(lambda kc=kc: vp_group(kc, 0))
                elif hp in (3, 4):
                    for kc in range(KC // 2):
                        kcv = kc + (0 if hp == 3 else KC // 2)
                        items.append(lambda kc=kcv: vp_group(kc, 1))
                return state, items

            def attn_items(state, hp):
                items = []

                def round_item(t, r):
                    qt_sb, kt_sb = state["qt_sb"], state["kt_sb"]
                    kc0 = 2 * r
                    if r == 0:
                        state[("avA", t)] = avp.tile([P, F], f32, tag="av", name="avA")
                        state[("avB", t)] = avp.tile([P, F], f32, tag="av", name="avB")
                    avA, avB = state[("avA", t)], state[("avB", t)]
                    st = stp.tile([P, 4, F], f32, tag="st", name="st")
                    # head-pair packed: A rows 0-63, B rows 64-127
                    for i, kc in ((0, kc0), (1, kc0 + 1)):
                        nc.tensor.matmul(
                            st[:, 2 * i, :],
                            kt_sb[0:DH, ts(kc, P)],
                            qt_sb[0:DH, ts(t, F)],
                            start=True,
                            stop=True,
                        )
                        nc.tensor.matmul(
                            st[:, 2 * i + 1, :],
                            kt_sb[DH:P, ts(kc, P)],
                            qt_sb[DH:P, ts(t, F)],
                            start=True,
                            stop=True,
                        )
                    at = apool.tile([P, 4, F], fp16, tag="attn", name="at")
                    nc.scalar.activation(at[:], st[:], EXP, scale=SCALE)
                    for i, kc in ((0, kc0), (1, kc0 + 1)):
                        nc.tensor.matmul(
                            avA[0:65, :],
                            vp_sbs[kc][:, hp, 0:65],
                            at[:, 2 * i, :],
                            start=(kc == 0),
                            stop=(kc == KC - 1),
                        )
                        nc.tensor.matmul(
                            avB[0:65, :],
                            vp_sbs[kc][:, hp, 65:130],
                            at[:, 2 * i + 1, :],
                            start=(kc == 0),
                            stop=(kc == KC - 1),
                        )

                def normalize(t, h):
                    av = state[("avA", t)] if h == 0 else state[("avB", t)]
                    s = 2 * hp + h
                    rd = npool.tile([P, F], fp16, tag="rd", name="rd")
                    with nc.allow_low_precision("softmax denom recip, ~5e-4 ok"):
                        nc.vector.reciprocal(rd[64:65, :], av[64:65, :])
                    bps = bcp.tile([DH, F], f32, tag="bps", name="bps")
                    nc.tensor.matmul(
                        bps[:],
                        ones_sb[64:65, :],
                        rd[64:65, :],
                        start=True,
                        stop=True,
                    )
                    bc = npool.tile([DH, F], fp16, tag="bc", name="bc")
                    nc.vector.tensor_copy(bc[:], bps[:])
                    nc.vector.tensor_mul(ot_sb[:, s, ts(t, F)], av[0:DH, :], bc[:])

                for t in range(QT):
                    for r in range(KC // 2):
                        items.append(lambda t=t, r=r: round_item(t, r))
                    items.append(lambda t=t: normalize(t, 0))
                    items.append(lambda t=t: normalize(t, 1))
                return items

            def y_items():
                items = []
                state = {}

                def y_group_t(dc, t):
                    wo_t = wpool.tile([DH, H, P], fp16, tag="wo", name="wo_t")
                    nc.sync.dma_start(wo_t[:], wo[:, dc])
                    ysb = npool.tile([P, F], f32, tag="y", name="ysb", bufs=2)
                    ps = mmp.tile([P, F], f32, tag="proj", name="ps")
                    for s in range(H):
                        nc.tensor.matmul(
                            ps[:],
                            wo_t[:, s, :],
                            ot_sb[:, s, ts(t, F)],
                            start=(s == 0),
                            stop=(s == H - 1),
                        )
                    nc.vector.tensor_copy(ysb[:], ps[:])
                    nc.sync.dma_start(yT[:, dc, ts(t, F)], ysb[:])

                # emission-order safety: a read emitted before its writer
                # gets no RAW edge, so a Y group may only be emitted after
                # ALL normalize items writing the ot slices it reads
                g_t0 = [lambda dc=dc: y_group_t(dc, 0) for dc in range(CC)]
                g_t1 = [lambda dc=dc: y_group_t(dc, 1) for dc in range(CC)]
                return g_t0, g_t1

            for _rep in range(repeat):
                prev_attn = []
                for hp in range(HP):
                    state, pitems = proj_items(hp)
                    if _rep == 0 and hp == 0:
                        pitems[0]()  # weight DMAs for hp 0 queue first
                        load_x()
                        pitems = pitems[1:]
                    _interleave(prev_attn, pitems)
                    prev_attn = attn_items(state, hp)
                # attention items per hp: indices 0-9 are the qt=0 rounds
                # and its two normalizes; 10-19 are the qt=1 half. qt0 Y
                # groups interleave with the qt1 half (their ot writers are
                # all emitted by then); qt1 Y groups go last.
                y_g0, y_g1 = y_items()
                for it in prev_attn[:10]:
                    it()
                _interleave(prev_attn[10:], y_g0)
                for g in y_g1:
                    g()

    nc.compile()
    return nc


_NC_CACHE = None


def _get_nc():
    global _NC_CACHE
    if _NC_CACHE is None:
        _NC_CACHE = build_nc()
    return _NC_CACHE


def _chunk_T(a):
    # [rows, D] f32 -> [P, D//P, rows] fp16 (feature-major chunks)
    return np.ascontiguousarray(
        a.T.reshape(CC, P, a.shape[0]).transpose(1, 0, 2).astype(np.float16)
    )


def _prep_in_maps(query, key, value, Wq, bq, Wk, bk, Wv, bv, Wo, bo):
    wq_d = np.ascontiguousarray(
        Wq.reshape(HP, P, CC, P).transpose(3, 0, 2, 1).astype(np.float16)
    )
    wk_d = np.ascontiguousarray(
        Wk.reshape(HP, P, CC, P).transpose(3, 0, 2, 1).astype(np.float16)
    )
    # wv: [p, c, j] = Wv[j, 128c+p] (dout-major for 512-wide V' groups)
    wv_d = np.ascontiguousarray(
        Wv.reshape(D, CC, P).transpose(2, 1, 0).astype(np.float16)
    )
    wo_d = np.ascontiguousarray(
        Wo.reshape(CC, P, H, DH).transpose(3, 0, 2, 1).astype(np.float16)
    )
    bq_d = np.ascontiguousarray(bq.reshape(HP, P).T.astype(np.float32))
    bk_d = np.ascontiguousarray(bk.reshape(HP, P).T.astype(np.float32))

    in_maps = []
    for c in range(N_CORES):
        b, th = divmod(c, 2)
        in_maps.append(
            {
                "xq": _chunk_T(query[b, th * TQ : (th + 1) * TQ, :]),
                "xk": _chunk_T(key[b]),
                "xv": _chunk_T(value[b]),
                "wq": wq_d,
                "wk": wk_d,
                "wv": wv_d,
                "wo": wo_d,
                "bq": bq_d,
                "bk": bk_d,
            }
        )
    return in_maps


def _gather(results, bv, bo, Wo):
    crow = (bv.astype(np.float32) @ Wo.T.astype(np.float32) + bo).astype(np.float32)
    out = np.empty((B, T, D), np.float32)
    for c in range(N_CORES):
        b, th = divmod(c, 2)
        y = results[c]["yT"]  # [P, CC, TQ] f32 = Y^T chunks
        out[b, th * TQ : (th + 1) * TQ, :] = (
            y.transpose(1, 0, 2).reshape(D, TQ).T + crow
        )
    return out


def _run(inputs, trace=False, **kwargs):
    inputs = {k: np.asarray(v) for k, v in inputs.items()}
    nc = _get_nc()
    in_maps = _prep_in_maps(**inputs)
    res = bass_utils.run_bass_kernel_spmd(
        nc, in_maps, core_ids=list(range(N_CORES)), trace=trace, **kwargs
    )
    out = _gather(res.results, inputs["bv"], inputs["bo"], inputs["Wo"])
    return out, res


def kernel(**inputs):
    out, _ = _run(inputs, trace=False)
    return out
